# revision 12
# baseline (speedup 1.0000x reference)
"""Trainium2 Bass kernel for the Luong attention layer.

reference:
    score = einsum('bsh,bth->bst', enc, dec)        # [B,S,T]
    attn  = softmax(score, axis=1)                  # over S
    ev    = einsum('bst,bsh->bth', attn, enc)       # [B,T,H]
    out   = concat([dec, ev], axis=-1)              # [B,T,2H]

Strategy: data-parallel over B (16 batches -> 8 cores x 2). Per batch on
device:
    scoreT[t,s] = sum_h decT[h,t] * encT[h,s]   (PE, lhsT=decT block, rhs=encT)
    softmax over free dim s: reduce_max(negate) -> Exp(bias=-max, accum_out=Z)
    attn[s,t] blocks = PE-transpose of exp tiles
    ev[t,h] = sum_s attn[s,t] * enc[s,h]        (PE, lhsT=attn block, rhs=enc)
    evacuate ev with scale=1/Z (per-t scalar) on ScalarE
Host pre-transposes enc/dec to [H,*] layout (layout prep) and assembles
out = concat([dec, ev]) when unsharding.

Precision modes (ATTN_KERNEL_MODE):
    bf16x3 (default): mm1 = 3-pass bf16 hi/lo split
        score ~= hiT.hi + hiT.lo + loT.hi  (per-product err ~2^-18)
        mm2 = f32r (1 cyc/row, ~11-bit multiply, attn in [0,1])
    f32r: both matmuls f32r (fast, score err ~2e-2 abs -> ~2% attn err)
    f32:  both matmuls fp32 (4 cyc/row, exact)
"""

import os
import sys

if "/opt/trn_rl_repo" not in sys.path:
    sys.path.insert(0, "/opt/trn_rl_repo")

import numpy as np

B, S, T, H = 16, 1024, 1024, 1024
NCORES = 8
BLOC = B // NCORES  # batches per core
P = 128
NT = S // P  # 8 tiles along each 1024 dim
NCH = 2  # 512-wide moving chunks per 1024
CH = 512

MODE = os.environ.get("ATTN_KERNEL_MODE", "bf16x3")
# timing aid: >1 wraps the whole computation in a hardware For_i loop
LOOP = int(os.environ.get("ATTN_KERNEL_LOOP", "1"))

_prog_cache = {}
last_results = None  # stash for test harness introspection


def _build_program(mode, loop=1):
    from concourse import bacc
    import concourse.mybir as mybir
    import concourse.tile as tile
    from concourse.masks import make_identity

    dt = mybir.dt
    AF = mybir.ActivationFunctionType
    AX = mybir.AxisListType

    split = mode == "bf16x3"
    mm1_dt = dt.bfloat16 if split else (dt.float32r if mode == "f32r" else dt.float32)
    mm2_dt = dt.float32r if mode != "f32" else dt.float32

    nc = bacc.Bacc("TRN2", target_bir_lowering=False, debug=False)

    if split:
        enc_t_hi = nc.dram_tensor(
            "enc_t_hi", [BLOC, H, S], dt.bfloat16, kind="ExternalInput"
        ).ap()
        enc_t_lo = nc.dram_tensor(
            "enc_t_lo", [BLOC, H, S], dt.bfloat16, kind="ExternalInput"
        ).ap()
        dec_t_hi = nc.dram_tensor(
            "dec_t_hi", [BLOC, H, T], dt.bfloat16, kind="ExternalInput"
        ).ap()
        dec_t_lo = nc.dram_tensor(
            "dec_t_lo", [BLOC, H, T], dt.bfloat16, kind="ExternalInput"
        ).ap()
    else:
        enc_t = nc.dram_tensor(
            "enc_t", [BLOC, H, S], dt.float32, kind="ExternalInput"
        ).ap().bitcast(mm1_dt)
        dec_t = nc.dram_tensor(
            "dec_t", [BLOC, H, T], dt.float32, kind="ExternalInput"
        ).ap().bitcast(mm1_dt)
    enc_n = nc.dram_tensor(
        "enc_n", [BLOC, S, H], dt.float32, kind="ExternalInput"
    ).ap().bitcast(mm2_dt)
    ev = nc.dram_tensor("ev", [BLOC, T, H], dt.float32, kind="ExternalOutput").ap()

    with tile.TileContext(nc) as tc:
        with (
            tc.tile_pool(name="const", bufs=1) as const_pool,
            tc.tile_pool(name="big", bufs=2) as big_pool,
            tc.tile_pool(name="dec_blk", bufs=3) as dec_pool,
            tc.tile_pool(name="work", bufs=3) as work_pool,
            tc.tile_pool(name="attn", bufs=2) as attn_pool,
            tc.tile_pool(name="stats", bufs=4) as stats_pool,
            tc.tile_pool(name="ps_score", bufs=2, space="PSUM") as ps_score_pool,
            tc.tile_pool(name="ps_ev", bufs=1, space="PSUM") as ps_ev_pool,
            tc.tile_pool(name="ps_tr", bufs=2, space="PSUM") as ps_tr_pool,
        ):
            ident = const_pool.tile([P, P], dt.float32)
            make_identity(nc, ident)

            import contextlib

            loop_cm = tc.For_i(0, loop, 1) if loop > 1 else contextlib.nullcontext()
            with loop_cm:
                _emit_body(
                    nc,
                    tc,
                    dt,
                    AF,
                    AX,
                    split,
                    mm1_dt,
                    mm2_dt,
                    locals_in := dict(
                        big_pool=big_pool,
                        dec_pool=dec_pool,
                        work_pool=work_pool,
                        attn_pool=attn_pool,
                        stats_pool=stats_pool,
                        ps_score_pool=ps_score_pool,
                        ps_ev_pool=ps_ev_pool,
                        ps_tr_pool=ps_tr_pool,
                        ident=ident,
                        enc_t_hi=enc_t_hi if split else None,
                        enc_t_lo=enc_t_lo if split else None,
                        dec_t_hi=dec_t_hi if split else None,
                        dec_t_lo=dec_t_lo if split else None,
                        enc_t=None if split else enc_t,
                        dec_t=None if split else dec_t,
                        enc_n=enc_n,
                        ev=ev,
                    ),
                )

    nc.finalize()
    return nc


def _emit_body(nc, tc, dt, AF, AX, split, mm1_dt, mm2_dt, env):
    big_pool = env["big_pool"]
    dec_pool = env["dec_pool"]
    work_pool = env["work_pool"]
    attn_pool = env["attn_pool"]
    stats_pool = env["stats_pool"]
    ps_score_pool = env["ps_score_pool"]
    ps_ev_pool = env["ps_ev_pool"]
    ps_tr_pool = env["ps_tr_pool"]
    ident = env["ident"]
    enc_t_hi = env["enc_t_hi"]
    enc_t_lo = env["enc_t_lo"]
    dec_t_hi = env["dec_t_hi"]
    dec_t_lo = env["dec_t_lo"]
    enc_t = env["enc_t"]
    dec_t = env["dec_t"]
    enc_n = env["enc_n"]
    ev = env["ev"]

    if True:
        if True:
            for b in range(BLOC):
                # batch-persistent arrays, [128, k, 1024] layout
                if split:
                    encT_hi_sb = big_pool.tile([P, NT, S], dt.bfloat16, tag="encT_hi")
                    nc.sync.dma_start(
                        encT_hi_sb, enc_t_hi[b].rearrange("(k p) s -> p k s", p=P)
                    )
                    encT_lo_sb = big_pool.tile([P, NT, S], dt.bfloat16, tag="encT_lo")
                    nc.sync.dma_start(
                        encT_lo_sb, enc_t_lo[b].rearrange("(k p) s -> p k s", p=P)
                    )
                else:
                    encT_sb = big_pool.tile([P, NT, S], mm1_dt, tag="encT")
                    nc.sync.dma_start(
                        encT_sb, enc_t[b].rearrange("(k p) s -> p k s", p=P)
                    )
                encN_sb = big_pool.tile([P, NT, H], mm2_dt, tag="encN")
                nc.sync.dma_start(
                    encN_sb, enc_n[b].rearrange("(j p) h -> p j h", p=P)
                )

                for i in range(NT):  # t-tile
                    ti = slice(i * P, (i + 1) * P)
                    # stationary blocks for mm1: [:, k, :] = [K=128 h, M=128 t]
                    if split:
                        decT_hi_blk = dec_pool.tile(
                            [P, NT, P], dt.bfloat16, tag="decT_hi"
                        )
                        nc.sync.dma_start(
                            decT_hi_blk,
                            dec_t_hi[b][:, ti].rearrange("(k p) t -> p k t", p=P),
                        )
                        decT_lo_blk = dec_pool.tile(
                            [P, NT, P], dt.bfloat16, tag="decT_lo"
                        )
                        nc.sync.dma_start(
                            decT_lo_blk,
                            dec_t_lo[b][:, ti].rearrange("(k p) t -> p k t", p=P),
                        )
                        # (lhsT, rhs) per pass: hi.hi + hi.lo + lo.hi
                        passes = [
                            (decT_hi_blk, encT_hi_sb),
                            (decT_hi_blk, encT_lo_sb),
                            (decT_lo_blk, encT_hi_sb),
                        ]
                    else:
                        decT_blk = dec_pool.tile([P, NT, P], mm1_dt, tag="decT")
                        nc.sync.dma_start(
                            decT_blk,
                            dec_t[b][:, ti].rearrange("(k p) t -> p k t", p=P),
                        )
                        passes = [(decT_blk, encT_sb)]

                    # ---- mm1: scoreT[t_i, s] ----
                    ps_score = ps_score_pool.tile([P, S], dt.float32, tag="score")
                    npass = len(passes)
                    for n in range(NCH):
                        for ip, (lhsT, rhs) in enumerate(passes):
                            for k in range(NT):
                                nc.tensor.matmul(
                                    ps_score[:, n * CH : (n + 1) * CH],
                                    lhsT[:, k, :],
                                    rhs[:, k, n * CH : (n + 1) * CH],
                                    start=(ip == 0 and k == 0),
                                    stop=(ip == npass - 1 and k == NT - 1),
                                )

                    # ---- softmax over s (free dim) ----
                    neg_max = stats_pool.tile([P, 1], dt.float32, tag="negmax")
                    nc.vector.reduce_max(
                        out=neg_max, in_=ps_score, axis=AX.X, negate=True
                    )
                    sumexp = stats_pool.tile([P, 1], dt.float32, tag="sumexp")
                    exp_sb = work_pool.tile([P, S], dt.float32, tag="exp")
                    nc.scalar.activation(
                        out=exp_sb,
                        in_=ps_score,
                        func=AF.Exp,
                        bias=neg_max,
                        accum_out=sumexp,
                    )
                    recip = stats_pool.tile([P, 1], dt.float32, tag="recip")
                    nc.vector.reciprocal(recip, sumexp)

                    # ---- transpose exp[t_i, s] -> attn blocks [s_j, t_i] ----
                    attn_sb = attn_pool.tile([P, NT, P], mm2_dt, tag="attn")
                    for j in range(NT):
                        ps_tr = ps_tr_pool.tile([P, P], dt.float32, tag="tr")
                        nc.tensor.transpose(
                            ps_tr, exp_sb[:, j * P : (j + 1) * P], ident
                        )
                        nc.vector.tensor_copy(attn_sb[:, j, :], ps_tr)

                    # ---- mm2: ev[t_i, h] = sum_s attn[s,t_i] enc[s,h] ----
                    ps_ev = ps_ev_pool.tile([P, H], dt.float32, tag="ev")
                    for n in range(NCH):
                        for k in range(NT):
                            nc.tensor.matmul(
                                ps_ev[:, n * CH : (n + 1) * CH],
                                attn_sb[:, k, :],
                                encN_sb[:, k, n * CH : (n + 1) * CH],
                                start=(k == 0),
                                stop=(k == NT - 1),
                            )

                    # ---- evacuate + normalize by 1/Z ----
                    ev_sb = work_pool.tile([P, H], dt.float32, tag="evout")
                    nc.scalar.mul(ev_sb, ps_ev, recip)
                    nc.sync.dma_start(ev[b, ti, :], ev_sb)


def _get_program(mode, loop=1):
    key = (mode, loop)
    if key not in _prog_cache:
        _prog_cache[key] = _build_program(mode, loop)
    return _prog_cache[key]


def _bf16_split(x):
    import ml_dtypes

    hi = x.astype(ml_dtypes.bfloat16)
    lo = (x - hi.astype(np.float32)).astype(ml_dtypes.bfloat16)
    return hi, lo


def kernel(encoder_outputs, decoder_outputs):
    global last_results
    from concourse.bass_utils import run_bass_kernel_spmd

    enc = np.ascontiguousarray(np.asarray(encoder_outputs, dtype=np.float32))
    dec = np.ascontiguousarray(np.asarray(decoder_outputs, dtype=np.float32))
    assert enc.shape == (B, S, H) and dec.shape == (B, T, H)

    split = MODE == "bf16x3"
    in_maps = []
    for c in range(NCORES):
        e = enc[c * BLOC : (c + 1) * BLOC]
        d = dec[c * BLOC : (c + 1) * BLOC]
        et = np.ascontiguousarray(e.transpose(0, 2, 1))
        dtp = np.ascontiguousarray(d.transpose(0, 2, 1))
        m = {"enc_n": e}
        if split:
            m["enc_t_hi"], m["enc_t_lo"] = _bf16_split(et)
            m["dec_t_hi"], m["dec_t_lo"] = _bf16_split(dtp)
        else:
            m["enc_t"] = et
            m["dec_t"] = dtp
        in_maps.append(m)

    nc = _get_program(MODE, LOOP)
    trace = bool(int(os.environ.get("ATTN_KERNEL_TRACE", "0")))
    last_results = run_bass_kernel_spmd(
        nc, in_maps, core_ids=list(range(NCORES)), trace=trace
    )
    ev_full = np.concatenate(
        [last_results.results[c]["ev"] for c in range(NCORES)], axis=0
    )
    return np.concatenate([dec, ev_full], axis=-1)
